# revision 16
# baseline (speedup 1.0000x reference)
"""GQA kernel for Trainium2, 8 NeuronCores.

Sharding: core c = b*4 + g handles batch b, kv-head g (4 query heads).
Host sums the 4 partial outputs per batch.

Per core (all matmuls bf16, f32 PSUM):
  Q_h^T = Wq_h @ x_q^T   [128 d, S]
  K^T   = Wk  @ x_k^T    [128 d, S]
  V     = [S, 128]  (V^T via matmul, then DMA-XBAR transpose)
  S^T   = K_tile @ Q^T -> [k, q] scores; exp on scalar engine into
          bf16 P; causal diagonal handled by column-narrowed scores
          matmul + narrowed exp + gpsimd memset (fully-masked cols)
          + gpsimd triangle multiply (keeps DVE/scalar free)
  o^T  += V[kt] @ P[kt] (PSUM), l += ones @ P[kt]
  norm  = reciprocal_approx_fast(l) [1,SC] -> gpsimd
          partition_broadcast -> DVE mul out of PSUM
  out   = onrm @ Wo -> bf16 partial [S, E]

Perf notes vs the 477us baseline:
 - The PE clock p-states ramp 0.65 -> 1.2 -> 2.4 GHz and reach full
   speed only after ~3us of gap-free execution, so emission is
   software-pipelined: per attention head we interleave the next
   head's Q projection and the previous chunk's outproj stripes so
   the PE queue never starves while exps run.
 - The l reciprocal runs on [1,SC] via reciprocal_approx_fast (~5x
   cheaper than InstReciprocal on the broadcast [128,SC]).
 - DMAs are batched (one 3D-AP descriptor per x chunk / weight) and
   spread across the sync queue (x, V transposes) and gpsimd SWDGE
   (weights, l bounce, output writes) so no single queue serializes
   and DMA issue stays off the scalar/vector engines.
 - exp work on the causal diagonal is column-narrowed (~12% less
   scalar time); triangle masking runs on the idle gpsimd engine as
   a 0/1 multiply after exp.
"""

import sys

import numpy as np

for _p in ("/opt/trn_rl_repo",):
    if _p not in sys.path:
        sys.path.insert(0, _p)

import ml_dtypes

import concourse.bass as bass
import concourse.mybir as mybir
from concourse import bacc
from concourse.bass_utils import run_bass_kernel_spmd
from concourse.tile import TileContext

B, S, E = 2, 2048, 2048
H, HKV = 16, 4
D = E // H  # 128
G = H // HKV  # 4 query heads per kv head
GD = G * D  # 512
NCORES = B * HKV  # 8
SC = 512  # s/q chunk width (free dim of matmuls)
NSC = S // SC  # 4
NET = E // 128  # 16 e-tiles (contraction)
NKT = S // 128  # 16 k-tiles
SCALE = 1.0 / float(np.sqrt(D))

F32 = mybir.dt.float32
BF16 = mybir.dt.bfloat16
AF = mybir.ActivationFunctionType
NPBF = np.dtype(ml_dtypes.bfloat16)


def build_nc():
    nc = bacc.Bacc()
    xq = nc.declare_dram_parameter("xq", [E, S], BF16, isOutput=False)  # query[b].T
    xk = nc.declare_dram_parameter("xk", [E, S], BF16, isOutput=False)  # key[b].T
    xv = nc.declare_dram_parameter("xv", [E, S], BF16, isOutput=False)  # value[b].T
    wq = nc.declare_dram_parameter("wq", [E, GD], BF16, isOutput=False)
    wk = nc.declare_dram_parameter("wk", [E, D], BF16, isOutput=False)
    wv = nc.declare_dram_parameter("wv", [E, D], BF16, isOutput=False)
    wo = nc.declare_dram_parameter("wo", [GD, E], BF16, isOutput=False)
    msk = nc.declare_dram_parameter("msk", [128, 128], BF16, isOutput=False)
    out = nc.declare_dram_parameter("out", [S, E], BF16, isOutput=True)

    def tiled3(dram, ncols, tiles, coff=0, width=None):
        # [128, tiles, width] gather of row-tiled [tiles*128, ncols] DRAM
        w = ncols if width is None else width
        base = dram[:, :]
        return bass.AP(
            tensor=base.tensor,
            offset=coff,
            ap=[[ncols, 128], [128 * ncols, tiles], [1, w]],
        )

    with TileContext(nc) as tc:
        with (
            tc.tile_pool(name="singles", bufs=1) as singles,
            tc.tile_pool(name="xt", bufs=5) as xtp,
            tc.tile_pool(name="pexp", bufs=4) as pexp,
            tc.tile_pool(name="vt", bufs=2) as vtp,
            tc.tile_pool(name="rl", bufs=2) as rlp,
            tc.tile_pool(name="rb", bufs=2) as rbp,
            tc.tile_pool(name="ob", bufs=2) as obp,
            tc.tile_pool(name="acc", bufs=5, space="PSUM") as acc,
            tc.tile_pool(name="ops", bufs=2, space="PSUM") as ops,
            tc.tile_pool(name="lps", bufs=1, space="PSUM") as lps,
        ):
            # ---- constants / weights resident in SBUF ----
            wq_sb = singles.tile([128, NET, GD], BF16)  # 16KB/p
            wk_sb = singles.tile([128, NET, D], BF16)  # 4KB/p
            wv_sb = singles.tile([128, NET, D], BF16)  # 4KB/p
            wo_sb = singles.tile([128, G, E], BF16)  # 16KB/p
            tri = singles.tile([128, 128], BF16)  # [k, q] 1=attend
            ones1 = singles.tile([128, 1], BF16)
            qT = singles.tile([128, G, S], BF16)  # 16KB/p
            kT = singles.tile([128, S], BF16)  # 4KB/p
            v_sb = singles.tile([128, NKT, D], BF16)  # 4KB/p
            onrm = singles.tile([128, G, S], BF16)  # 16KB/p

            nc.vector.memset(ones1, 1.0)

            xts = {}

            def load_chunk(sc, eng=None):
                eng = eng or {}
                for nm, dram in (("xk", xk), ("xv", xv), ("xq", xq)):
                    t = xtp.tile([128, NET, SC], BF16, tag="xt")
                    eng.get(nm, nc.sync).dma_start(
                        out=t[:],
                        in_=tiled3(dram, S, NET, coff=sc * SC, width=SC),
                    )
                    xts[(nm, sc)] = t

            # startup: split the first chunk across the sync queue (xk, xq)
            # and the gpsimd swdge queue (weights, xv) so both queues fill
            # the HBM pipe in parallel; order follows first use.
            t0 = xtp.tile([128, NET, SC], BF16, tag="xt")
            nc.sync.dma_start(
                out=t0[:, : NET // 2, :], in_=tiled3(xk, S, NET // 2, width=SC)
            )
            nc.sync.dma_start(
                out=t0[:, NET // 2 :, :],
                in_=bass.AP(
                    tensor=xk[:, :].tensor,
                    offset=(NET // 2) * 128 * S,
                    ap=[[S, 128], [128 * S, NET // 2], [1, SC]],
                ),
            )
            xts[("xk", 0)] = t0
            nc.gpsimd.dma_start(out=wk_sb[:], in_=tiled3(wk, D, NET))
            t0 = xtp.tile([128, NET, SC], BF16, tag="xt")
            nc.gpsimd.dma_start(out=t0[:], in_=tiled3(xv, S, NET, width=SC))
            xts[("xv", 0)] = t0
            t0 = xtp.tile([128, NET, SC], BF16, tag="xt")
            nc.sync.dma_start(out=t0[:], in_=tiled3(xq, S, NET, width=SC))
            xts[("xq", 0)] = t0
            nc.gpsimd.dma_start(out=wv_sb[:], in_=tiled3(wv, D, NET))
            nc.gpsimd.dma_start(out=wq_sb[:], in_=tiled3(wq, GD, NET))
            nc.gpsimd.dma_start(out=tri[:], in_=msk[:, :])
            nc.gpsimd.dma_start(out=wo_sb[:], in_=tiled3(wo, E, G))

            def proj(w_sb, x_t, out_ps, hslice):
                for t in range(NET):
                    nc.tensor.matmul(
                        out_ps[:],
                        lhsT=w_sb[:, t, hslice],
                        rhs=x_t[:, t, :],
                        start=(t == 0),
                        stop=(t == NET - 1),
                    )

            def emit_outproj_stripe(c, sti):
                st = c * (SC // 128) + sti
                stl = slice(st * 128, (st + 1) * 128)
                ob = obp.tile([128, E], BF16, tag="ob")
                for ec in range(E // SC):
                    esl = slice(ec * SC, (ec + 1) * SC)
                    ps = acc.tile([128, SC], F32, tag="acc")
                    for h in range(G):
                        nc.tensor.matmul(
                            ps[:],
                            lhsT=onrm[:, h, stl],
                            rhs=wo_sb[:, h, esl],
                            start=(h == 0),
                            stop=(h == G - 1),
                        )
                    nc.vector.tensor_copy(out=ob[:, esl], in_=ps[:])
                nc.gpsimd.dma_start(out=out[stl, :], in_=ob[:])

            def emit_normalize(h, ssl):
                # 1/l on [1,SC]; gpsimd partition-broadcast across the 128
                # partitions (no DRAM bounce); multiply out of PSUM.
                o_ps, l_ps = olps[h]
                rl = rlp.tile([1, SC], F32, tag="rl")
                nc.vector.reciprocal_approx_fast(out=rl[:], in_=l_ps[:])
                rb = rbp.tile([128, SC], F32, tag="rb")
                nc.gpsimd.partition_broadcast(rb[:], rl[:])
                nc.vector.tensor_mul(onrm[:, h, ssl], o_ps[:], rb[:])

            olps = {}
            for sc in range(NSC):
                ssl = slice(sc * SC, (sc + 1) * SC)
                if sc + 1 < NSC:
                    load_chunk(sc + 1)
                # K projection
                ps = acc.tile([128, SC], F32, tag="acc")
                proj(wk_sb, xts[("xk", sc)], ps, slice(0, D))
                nc.vector.tensor_copy(out=kT[:, ssl], in_=ps[:])
                # V projection + transpose to [s, d]
                ps = acc.tile([128, SC], F32, tag="acc")
                proj(wv_sb, xts[("xv", sc)], ps, slice(0, D))
                vt = vtp.tile([128, SC], BF16, tag="vt")
                nc.scalar.activation(out=vt[:], in_=ps[:], func=AF.Copy)
                for i in range(SC // 128):
                    # transpose off the PE via the DMA XBAR; the scalar
                    # hwdge queue is otherwise empty so these never sit
                    # behind the bulk x transfers
                    nc.scalar.dma_start(
                        out=v_sb[:, sc * 4 + i, :],
                        in_=vt[:, i * 128 : (i + 1) * 128],
                        transpose=True,
                    )

                nkt = (sc + 1) * (SC // 128)  # causal: k tiles 0..nkt-1
                for h in range(G):
                    # Q projection for this head
                    ps = acc.tile([128, SC], F32, tag="acc")
                    proj(wq_sb, xts[("xq", sc)], ps, slice(h * D, (h + 1) * D))
                    nc.vector.tensor_copy(out=qT[:, h, ssl], in_=ps[:])

                    # deferred: normalize previous head + outproj stripe of
                    # the previous chunk (keeps the DVE queue clear of the
                    # DMA-bounce wait when the next head's scores start)
                    if h > 0:
                        emit_normalize(h - 1, ssl)
                        if sc > 0:
                            emit_outproj_stripe(sc - 1, h - 1)

                    # attention: scores run one k-tile ahead of AV/l
                    o_ps = ops.tile([128, SC], F32, tag="o")
                    l_ps = lps.tile([1, SC], F32, tag="l")
                    olps[h] = (o_ps, l_ps)
                    pps = [None] * nkt

                    def emit_scores(kt):
                        pp = pexp.tile([128, SC], BF16, tag="p")
                        pps[kt] = pp
                        jj = kt - (nkt - 4)
                        w0 = max(jj, 0) * 128  # first unmasked column
                        s_ps = acc.tile([128, SC], F32, tag="acc")
                        nc.tensor.matmul(
                            s_ps[:, w0:SC],
                            lhsT=kT[:, kt * 128 : (kt + 1) * 128],
                            rhs=qT[:, h, sc * SC + w0 : (sc + 1) * SC],
                            start=True,
                            stop=True,
                        )
                        nc.scalar.activation(
                            out=pp[:, w0:SC],
                            in_=s_ps[:, w0:SC],
                            func=AF.Exp,
                            scale=SCALE,
                        )
                        if jj >= 0:
                            dsl = slice(jj * 128, (jj + 1) * 128)
                            nc.gpsimd.tensor_mul(pp[:, dsl], pp[:, dsl], tri[:])

                    def emit_av(kt):
                        # masked (zero) columns of diagonal tiles are
                        # skipped: kt==0 is always full width so start=True
                        # initializes the whole PSUM region.
                        pp = pps[kt]
                        w0 = max(kt - (nkt - 4), 0) * 128
                        nc.tensor.matmul(
                            o_ps[:, w0:SC],
                            lhsT=v_sb[:, kt, :],
                            rhs=pp[:, w0:SC],
                            start=(kt == 0),
                            stop=(kt == nkt - 1),
                        )
                        nc.tensor.matmul(
                            l_ps[:, w0:SC],
                            lhsT=ones1[:],
                            rhs=pp[:, w0:SC],
                            start=(kt == 0),
                            stop=(kt == nkt - 1),
                        )

                    for kt in range(nkt):
                        emit_scores(kt)
                        if kt >= 1:
                            emit_av(kt - 1)
                    emit_av(nkt - 1)

                emit_normalize(G - 1, ssl)
                if sc > 0:
                    emit_outproj_stripe(sc - 1, G - 1)

            for sti in range(SC // 128):
                emit_outproj_stripe(NSC - 1, sti)
    nc.compile()
    return nc


_NC_CACHE = None


def _get_nc():
    global _NC_CACHE
    if _NC_CACHE is None:
        _NC_CACHE = build_nc()
    return _NC_CACHE


def _prep_in_maps(query, key, value, attn_mask, Wq, Wk, Wv, Wo):
    query = np.asarray(query, dtype=np.float32)
    key = np.asarray(key, dtype=np.float32)
    value = np.asarray(value, dtype=np.float32)
    Wq = np.asarray(Wq, dtype=np.float32)
    Wk = np.asarray(Wk, dtype=np.float32)
    Wv = np.asarray(Wv, dtype=np.float32)
    Wo = np.asarray(Wo, dtype=np.float32)
    am = np.asarray(attn_mask)

    xqT = [np.ascontiguousarray(query[b].T).astype(NPBF) for b in range(B)]
    xkT = [np.ascontiguousarray(key[b].T).astype(NPBF) for b in range(B)]
    xvT = [np.ascontiguousarray(value[b].T).astype(NPBF) for b in range(B)]

    # [k, q] multiplicative triangle for the diagonal 128x128 blocks
    m0 = np.asarray(am[0, 0, :128, :128], dtype=np.float32)  # [q, k]
    tri = np.ascontiguousarray(m0.T).astype(NPBF)

    in_maps = []
    for b in range(B):
        for g in range(HKV):
            in_maps.append(
                {
                    "xq": xqT[b],
                    "xk": xkT[b],
                    "xv": xvT[b],
                    "wq": np.ascontiguousarray(
                        Wq[g * GD : (g + 1) * GD, :].T
                    ).astype(NPBF),
                    "wk": np.ascontiguousarray(
                        Wk[g * D : (g + 1) * D, :].T
                    ).astype(NPBF),
                    "wv": np.ascontiguousarray(
                        Wv[g * D : (g + 1) * D, :].T
                    ).astype(NPBF),
                    "wo": np.ascontiguousarray(
                        Wo[:, g * GD : (g + 1) * GD].T
                    ).astype(NPBF),
                    "msk": tri,
                }
            )
    return in_maps


def _run(inputs, trace=False, **kw):
    nc = _get_nc()
    in_maps = _prep_in_maps(**inputs)
    res = run_bass_kernel_spmd(
        nc, in_maps, list(range(NCORES)), trace=trace, **kw
    )
    outs = [np.asarray(r["out"]) for r in res.results]
    full = np.empty((B, S, E), dtype=np.float32)
    for b in range(B):
        acc = outs[b * HKV].astype(np.float32)
        for g in range(1, HKV):
            acc = acc + outs[b * HKV + g].astype(np.float32)
        full[b] = acc
    return full, res


def kernel(**inputs):
    full, _ = _run(inputs, trace=False)
    return full


# revision 17
# speedup vs baseline: 1.0324x; 1.0324x over previous
"""GQA kernel for Trainium2, 8 NeuronCores.

Sharding: core c = b*4 + g handles batch b, kv-head g (4 query heads).
Host sums the 4 partial outputs per batch.

Per core (all matmuls bf16, f32 PSUM):
  Q_h^T = Wq_h @ x_q^T   [128 d, S]
  K^T   = Wk  @ x_k^T    [128 d, S]
  V     = [S, 128]  (V^T via matmul, then PE transpose)
  S^T   = K_tile @ Q^T -> [k, q] scores; exp on scalar engine into
          bf16 P; causal diagonal handled by column-narrowed scores
          matmul + narrowed exp + gpsimd memset (fully-masked cols)
          + gpsimd triangle multiply (keeps DVE/scalar free)
  o^T  += V[kt] @ P[kt] (PSUM), l += ones @ P[kt]
  norm  = reciprocal_approx_fast(l) [1,SC] -> gpsimd
          partition_broadcast -> DVE mul out of PSUM
  out   = onrm @ Wo -> bf16 partial [S, E]

Perf notes vs the 477us baseline:
 - The PE clock p-states ramp 0.65 -> 1.2 -> 2.4 GHz and reach full
   speed only after ~3us of gap-free execution, so emission is
   software-pipelined: per attention head we interleave the next
   head's Q projection and the previous chunk's outproj stripes so
   the PE queue never starves while exps run.
 - The l reciprocal runs on [1,SC] via reciprocal_approx_fast (~5x
   cheaper than InstReciprocal on the broadcast [128,SC]).
 - DMAs are batched (one 3D-AP descriptor per x chunk / weight) and
   spread across the sync queue (x, V transposes) and gpsimd SWDGE
   (weights, l bounce, output writes) so no single queue serializes
   and DMA issue stays off the scalar/vector engines.
 - exp work on the causal diagonal is column-narrowed (~12% less
   scalar time); triangle masking runs on the idle gpsimd engine as
   a 0/1 multiply after exp.
"""

import sys

import numpy as np

for _p in ("/opt/trn_rl_repo",):
    if _p not in sys.path:
        sys.path.insert(0, _p)

import ml_dtypes

import concourse.bass as bass
import concourse.mybir as mybir
from concourse import bacc
from concourse.bass_utils import run_bass_kernel_spmd
from concourse.masks import make_identity
from concourse.tile import TileContext

B, S, E = 2, 2048, 2048
H, HKV = 16, 4
D = E // H  # 128
G = H // HKV  # 4 query heads per kv head
GD = G * D  # 512
NCORES = B * HKV  # 8
SC = 512  # s/q chunk width (free dim of matmuls)
NSC = S // SC  # 4
NET = E // 128  # 16 e-tiles (contraction)
NKT = S // 128  # 16 k-tiles
SCALE = 1.0 / float(np.sqrt(D))

F32 = mybir.dt.float32
BF16 = mybir.dt.bfloat16
AF = mybir.ActivationFunctionType
NPBF = np.dtype(ml_dtypes.bfloat16)


def build_nc():
    nc = bacc.Bacc()
    xq = nc.declare_dram_parameter("xq", [E, S], BF16, isOutput=False)  # query[b].T
    xk = nc.declare_dram_parameter("xk", [E, S], BF16, isOutput=False)  # key[b].T
    xv = nc.declare_dram_parameter("xv", [E, S], BF16, isOutput=False)  # value[b].T
    wq = nc.declare_dram_parameter("wq", [E, GD], BF16, isOutput=False)
    wk = nc.declare_dram_parameter("wk", [E, D], BF16, isOutput=False)
    wv = nc.declare_dram_parameter("wv", [E, D], BF16, isOutput=False)
    wo = nc.declare_dram_parameter("wo", [GD, E], BF16, isOutput=False)
    msk = nc.declare_dram_parameter("msk", [128, 128], BF16, isOutput=False)
    out = nc.declare_dram_parameter("out", [S, E], BF16, isOutput=True)

    def tiled3(dram, ncols, tiles, coff=0, width=None):
        # [128, tiles, width] gather of row-tiled [tiles*128, ncols] DRAM
        w = ncols if width is None else width
        base = dram[:, :]
        return bass.AP(
            tensor=base.tensor,
            offset=coff,
            ap=[[ncols, 128], [128 * ncols, tiles], [1, w]],
        )

    with TileContext(nc) as tc:
        with (
            tc.tile_pool(name="singles", bufs=1) as singles,
            tc.tile_pool(name="xt", bufs=5) as xtp,
            tc.tile_pool(name="pexp", bufs=4) as pexp,
            tc.tile_pool(name="vt", bufs=2) as vtp,
            tc.tile_pool(name="rl", bufs=2) as rlp,
            tc.tile_pool(name="rb", bufs=2) as rbp,
            tc.tile_pool(name="ob", bufs=2) as obp,
            tc.tile_pool(name="acc", bufs=4, space="PSUM") as acc,
            tc.tile_pool(name="ops", bufs=2, space="PSUM") as ops,
            tc.tile_pool(name="lps", bufs=1, space="PSUM") as lps,
            tc.tile_pool(name="trp", bufs=1, space="PSUM") as trp,
        ):
            # ---- constants / weights resident in SBUF ----
            wq_sb = singles.tile([128, NET, GD], BF16)  # 16KB/p
            wk_sb = singles.tile([128, NET, D], BF16)  # 4KB/p
            wv_sb = singles.tile([128, NET, D], BF16)  # 4KB/p
            wo_sb = singles.tile([128, G, E], BF16)  # 16KB/p
            tri = singles.tile([128, 128], BF16)  # [k, q] 1=attend
            ident_f = singles.tile([128, 128], F32)
            ident = singles.tile([128, 128], BF16)
            ones1 = singles.tile([128, 1], BF16)
            qT = singles.tile([128, G, S], BF16)  # 16KB/p
            kT = singles.tile([128, S], BF16)  # 4KB/p
            v_sb = singles.tile([128, NKT, D], BF16)  # 4KB/p
            onrm = singles.tile([128, G, S], BF16)  # 16KB/p

            make_identity(nc, ident_f)
            nc.scalar.activation(out=ident[:], in_=ident_f[:], func=AF.Copy)
            nc.vector.memset(ones1, 1.0)

            xts = {}

            def load_chunk(sc, eng=None):
                eng = eng or {}
                for nm, dram in (("xk", xk), ("xv", xv), ("xq", xq)):
                    t = xtp.tile([128, NET, SC], BF16, tag="xt")
                    eng.get(nm, nc.sync).dma_start(
                        out=t[:],
                        in_=tiled3(dram, S, NET, coff=sc * SC, width=SC),
                    )
                    xts[(nm, sc)] = t

            # startup: split the first chunk across the sync queue (xk, xq)
            # and the gpsimd swdge queue (weights, xv) so both queues fill
            # the HBM pipe in parallel; order follows first use.
            t0 = xtp.tile([128, NET, SC], BF16, tag="xt")
            nc.sync.dma_start(
                out=t0[:, : NET // 2, :], in_=tiled3(xk, S, NET // 2, width=SC)
            )
            nc.sync.dma_start(
                out=t0[:, NET // 2 :, :],
                in_=bass.AP(
                    tensor=xk[:, :].tensor,
                    offset=(NET // 2) * 128 * S,
                    ap=[[S, 128], [128 * S, NET // 2], [1, SC]],
                ),
            )
            xts[("xk", 0)] = t0
            nc.gpsimd.dma_start(out=wk_sb[:], in_=tiled3(wk, D, NET))
            t0 = xtp.tile([128, NET, SC], BF16, tag="xt")
            nc.gpsimd.dma_start(out=t0[:], in_=tiled3(xv, S, NET, width=SC))
            xts[("xv", 0)] = t0
            t0 = xtp.tile([128, NET, SC], BF16, tag="xt")
            nc.sync.dma_start(out=t0[:], in_=tiled3(xq, S, NET, width=SC))
            xts[("xq", 0)] = t0
            nc.gpsimd.dma_start(out=wv_sb[:], in_=tiled3(wv, D, NET))
            nc.gpsimd.dma_start(out=wq_sb[:], in_=tiled3(wq, GD, NET))
            nc.gpsimd.dma_start(out=tri[:], in_=msk[:, :])
            nc.gpsimd.dma_start(out=wo_sb[:], in_=tiled3(wo, E, G))

            def proj(w_sb, x_t, out_ps, hslice):
                for t in range(NET):
                    nc.tensor.matmul(
                        out_ps[:],
                        lhsT=w_sb[:, t, hslice],
                        rhs=x_t[:, t, :],
                        start=(t == 0),
                        stop=(t == NET - 1),
                    )

            def emit_outproj_stripe(c, sti):
                st = c * (SC // 128) + sti
                stl = slice(st * 128, (st + 1) * 128)
                ob = obp.tile([128, E], BF16, tag="ob")
                for ec in range(E // SC):
                    esl = slice(ec * SC, (ec + 1) * SC)
                    ps = acc.tile([128, SC], F32, tag="acc")
                    for h in range(G):
                        nc.tensor.matmul(
                            ps[:],
                            lhsT=onrm[:, h, stl],
                            rhs=wo_sb[:, h, esl],
                            start=(h == 0),
                            stop=(h == G - 1),
                        )
                    nc.vector.tensor_copy(out=ob[:, esl], in_=ps[:])
                nc.gpsimd.dma_start(out=out[stl, :], in_=ob[:])

            def emit_normalize(h, ssl):
                # 1/l on [1,SC]; gpsimd partition-broadcast across the 128
                # partitions (no DRAM bounce); multiply out of PSUM.
                o_ps, l_ps = olps[h]
                rl = rlp.tile([1, SC], F32, tag="rl")
                nc.vector.reciprocal_approx_fast(out=rl[:], in_=l_ps[:])
                rb = rbp.tile([128, SC], F32, tag="rb")
                nc.gpsimd.partition_broadcast(rb[:], rl[:])
                nc.vector.tensor_mul(onrm[:, h, ssl], o_ps[:], rb[:])

            olps = {}
            for sc in range(NSC):
                ssl = slice(sc * SC, (sc + 1) * SC)
                if sc + 1 < NSC:
                    load_chunk(sc + 1)
                # K projection
                ps = acc.tile([128, SC], F32, tag="acc")
                proj(wk_sb, xts[("xk", sc)], ps, slice(0, D))
                nc.vector.tensor_copy(out=kT[:, ssl], in_=ps[:])
                # V projection + transpose to [s, d]
                ps = acc.tile([128, SC], F32, tag="acc")
                proj(wv_sb, xts[("xv", sc)], ps, slice(0, D))
                # vt copy on DVE: the scalar queue backs up ~20us of exps
                # at chunk boundaries and would stall the transposes
                vt = vtp.tile([128, SC], BF16, tag="vt")
                nc.vector.tensor_copy(out=vt[:], in_=ps[:])
                for i in range(SC // 128):
                    tp = trp.tile([128, 128], BF16, tag="tr")
                    nc.tensor.transpose(
                        tp[:], vt[:, i * 128 : (i + 1) * 128], ident[:]
                    )
                    nc.vector.tensor_copy(out=v_sb[:, sc * 4 + i, :], in_=tp[:])

                nkt = (sc + 1) * (SC // 128)  # causal: k tiles 0..nkt-1
                for h in range(G):
                    # Q projection for this head
                    ps = acc.tile([128, SC], F32, tag="acc")
                    proj(wq_sb, xts[("xq", sc)], ps, slice(h * D, (h + 1) * D))
                    nc.vector.tensor_copy(out=qT[:, h, ssl], in_=ps[:])

                    # deferred: normalize previous head + outproj stripe of
                    # the previous chunk (keeps the DVE queue clear of the
                    # DMA-bounce wait when the next head's scores start)
                    if h > 0:
                        emit_normalize(h - 1, ssl)
                        if sc > 0:
                            emit_outproj_stripe(sc - 1, h - 1)

                    # attention: scores run one k-tile ahead of AV/l
                    o_ps = ops.tile([128, SC], F32, tag="o")
                    l_ps = lps.tile([1, SC], F32, tag="l")
                    olps[h] = (o_ps, l_ps)
                    pps = [None] * nkt

                    def emit_scores(kt):
                        pp = pexp.tile([128, SC], BF16, tag="p")
                        pps[kt] = pp
                        jj = kt - (nkt - 4)
                        w0 = max(jj, 0) * 128  # first unmasked column
                        s_ps = acc.tile([128, SC], F32, tag="acc")
                        nc.tensor.matmul(
                            s_ps[:, w0:SC],
                            lhsT=kT[:, kt * 128 : (kt + 1) * 128],
                            rhs=qT[:, h, sc * SC + w0 : (sc + 1) * SC],
                            start=True,
                            stop=True,
                        )
                        nc.scalar.activation(
                            out=pp[:, w0:SC],
                            in_=s_ps[:, w0:SC],
                            func=AF.Exp,
                            scale=SCALE,
                        )
                        if jj >= 0:
                            dsl = slice(jj * 128, (jj + 1) * 128)
                            nc.gpsimd.tensor_mul(pp[:, dsl], pp[:, dsl], tri[:])

                    def emit_av(kt):
                        # masked (zero) columns of diagonal tiles are
                        # skipped: kt==0 is always full width so start=True
                        # initializes the whole PSUM region.
                        pp = pps[kt]
                        w0 = max(kt - (nkt - 4), 0) * 128
                        nc.tensor.matmul(
                            o_ps[:, w0:SC],
                            lhsT=v_sb[:, kt, :],
                            rhs=pp[:, w0:SC],
                            start=(kt == 0),
                            stop=(kt == nkt - 1),
                        )
                        nc.tensor.matmul(
                            l_ps[:, w0:SC],
                            lhsT=ones1[:],
                            rhs=pp[:, w0:SC],
                            start=(kt == 0),
                            stop=(kt == nkt - 1),
                        )

                    for kt in range(nkt):
                        emit_scores(kt)
                        if kt >= 1:
                            emit_av(kt - 1)
                    emit_av(nkt - 1)

                emit_normalize(G - 1, ssl)
                if sc > 0:
                    emit_outproj_stripe(sc - 1, G - 1)

            for sti in range(SC // 128):
                emit_outproj_stripe(NSC - 1, sti)
    nc.compile()
    return nc


_NC_CACHE = None


def _get_nc():
    global _NC_CACHE
    if _NC_CACHE is None:
        _NC_CACHE = build_nc()
    return _NC_CACHE


def _prep_in_maps(query, key, value, attn_mask, Wq, Wk, Wv, Wo):
    query = np.asarray(query, dtype=np.float32)
    key = np.asarray(key, dtype=np.float32)
    value = np.asarray(value, dtype=np.float32)
    Wq = np.asarray(Wq, dtype=np.float32)
    Wk = np.asarray(Wk, dtype=np.float32)
    Wv = np.asarray(Wv, dtype=np.float32)
    Wo = np.asarray(Wo, dtype=np.float32)
    am = np.asarray(attn_mask)

    xqT = [np.ascontiguousarray(query[b].T).astype(NPBF) for b in range(B)]
    xkT = [np.ascontiguousarray(key[b].T).astype(NPBF) for b in range(B)]
    xvT = [np.ascontiguousarray(value[b].T).astype(NPBF) for b in range(B)]

    # [k, q] multiplicative triangle for the diagonal 128x128 blocks
    m0 = np.asarray(am[0, 0, :128, :128], dtype=np.float32)  # [q, k]
    tri = np.ascontiguousarray(m0.T).astype(NPBF)

    in_maps = []
    for b in range(B):
        for g in range(HKV):
            in_maps.append(
                {
                    "xq": xqT[b],
                    "xk": xkT[b],
                    "xv": xvT[b],
                    "wq": np.ascontiguousarray(
                        Wq[g * GD : (g + 1) * GD, :].T
                    ).astype(NPBF),
                    "wk": np.ascontiguousarray(
                        Wk[g * D : (g + 1) * D, :].T
                    ).astype(NPBF),
                    "wv": np.ascontiguousarray(
                        Wv[g * D : (g + 1) * D, :].T
                    ).astype(NPBF),
                    "wo": np.ascontiguousarray(
                        Wo[:, g * GD : (g + 1) * GD].T
                    ).astype(NPBF),
                    "msk": tri,
                }
            )
    return in_maps


def _run(inputs, trace=False, **kw):
    nc = _get_nc()
    in_maps = _prep_in_maps(**inputs)
    res = run_bass_kernel_spmd(
        nc, in_maps, list(range(NCORES)), trace=trace, **kw
    )
    outs = [np.asarray(r["out"]) for r in res.results]
    full = np.empty((B, S, E), dtype=np.float32)
    for b in range(B):
        acc = outs[b * HKV].astype(np.float32)
        for g in range(1, HKV):
            acc = acc + outs[b * HKV + g].astype(np.float32)
        full[b] = acc
    return full, res


def kernel(**inputs):
    full, _ = _run(inputs, trace=False)
    return full


# revision 18
# speedup vs baseline: 1.3610x; 1.3183x over previous
"""GQA kernel for Trainium2, 8 NeuronCores.

Sharding: core c = b*4 + g handles batch b, kv-head g (4 query heads).
Host sums the 4 partial outputs per batch.

Per core (all matmuls bf16, f32 PSUM):
  Q_h^T = Wq_h @ x_q^T   [128 d, S]
  K^T   = Wk  @ x_k^T    [128 d, S]
  V     = [S, 128]  (V^T via matmul, then PE transpose)
  S^T   = K_tile @ Q^T -> [k, q] scores; exp on scalar engine into
          bf16 P; causal diagonal handled by column-narrowed scores
          matmul + narrowed exp + gpsimd memset (fully-masked cols)
          + gpsimd triangle multiply (keeps DVE/scalar free)
  o^T  += V[kt] @ P[kt] (PSUM), l += ones @ P[kt]
  norm  = reciprocal_approx_fast(l) [1,SC] -> DRAM bounce broadcast
          (gpsimd swdge queue) -> DVE mul out of PSUM
  out   = onrm @ Wo -> bf16 partial [S, E]

Perf notes vs the 477us baseline:
 - The PE clock p-states ramp 0.65 -> 1.2 -> 2.4 GHz and reach full
   speed only after ~3us of gap-free execution, so emission is
   software-pipelined: per attention head we interleave the next
   head's Q projection and the previous chunk's outproj stripes so
   the PE queue never starves while exps run.
 - The l reciprocal runs on [1,SC] via reciprocal_approx_fast (~5x
   cheaper than InstReciprocal on the broadcast [128,SC]).
 - DMAs are batched (one 3D-AP descriptor per x chunk / weight) and
   spread across the sync queue (x, V transposes) and gpsimd SWDGE
   (weights, l bounce, output writes) so no single queue serializes
   and DMA issue stays off the scalar/vector engines.
 - exp work on the causal diagonal is column-narrowed (~12% less
   scalar time); triangle masking runs on the idle gpsimd engine as
   a 0/1 multiply after exp.
"""

import sys

import numpy as np

for _p in ("/opt/trn_rl_repo",):
    if _p not in sys.path:
        sys.path.insert(0, _p)

import ml_dtypes

import concourse.bass as bass
import concourse.mybir as mybir
from concourse import bacc
from concourse.bass_utils import run_bass_kernel_spmd
from concourse.masks import make_identity
from concourse.tile import TileContext

B, S, E = 2, 2048, 2048
H, HKV = 16, 4
D = E // H  # 128
G = H // HKV  # 4 query heads per kv head
GD = G * D  # 512
NCORES = B * HKV  # 8
SC = 512  # s/q chunk width (free dim of matmuls)
NSC = S // SC  # 4
NET = E // 128  # 16 e-tiles (contraction)
NKT = S // 128  # 16 k-tiles
SCALE = 1.0 / float(np.sqrt(D))

F32 = mybir.dt.float32
BF16 = mybir.dt.bfloat16
AF = mybir.ActivationFunctionType
NPBF = np.dtype(ml_dtypes.bfloat16)


def build_nc():
    nc = bacc.Bacc()
    xq = nc.declare_dram_parameter("xq", [E, S], BF16, isOutput=False)  # query[b].T
    xk = nc.declare_dram_parameter("xk", [E, S], BF16, isOutput=False)  # key[b].T
    xv = nc.declare_dram_parameter("xv", [E, S], BF16, isOutput=False)  # value[b].T
    wq = nc.declare_dram_parameter("wq", [E, GD], BF16, isOutput=False)
    wk = nc.declare_dram_parameter("wk", [E, D], BF16, isOutput=False)
    wv = nc.declare_dram_parameter("wv", [E, D], BF16, isOutput=False)
    wo = nc.declare_dram_parameter("wo", [GD, E], BF16, isOutput=False)
    msk = nc.declare_dram_parameter("msk", [128, 128], BF16, isOutput=False)
    out = nc.declare_dram_parameter("out", [S, E], BF16, isOutput=True)

    def tiled3(dram, ncols, tiles, coff=0, width=None):
        # [128, tiles, width] gather of row-tiled [tiles*128, ncols] DRAM
        w = ncols if width is None else width
        base = dram[:, :]
        return bass.AP(
            tensor=base.tensor,
            offset=coff,
            ap=[[ncols, 128], [128 * ncols, tiles], [1, w]],
        )

    with TileContext(nc) as tc:
        with (
            tc.tile_pool(name="singles", bufs=1) as singles,
            tc.tile_pool(name="xt", bufs=5) as xtp,
            tc.tile_pool(name="pexp", bufs=4) as pexp,
            tc.tile_pool(name="vt", bufs=2) as vtp,
            tc.tile_pool(name="rl", bufs=2) as rlp,
            tc.tile_pool(name="rb", bufs=2) as rbp,
            tc.tile_pool(name="ob", bufs=2) as obp,
            tc.tile_pool(name="acc", bufs=4, space="PSUM") as acc,
            tc.tile_pool(name="ops", bufs=2, space="PSUM") as ops,
            tc.tile_pool(name="lps", bufs=1, space="PSUM") as lps,
            tc.tile_pool(name="trp", bufs=1, space="PSUM") as trp,
            tc.tile_pool(name="drp", bufs=2, space="DRAM") as drp,
        ):
            # ---- constants / weights resident in SBUF ----
            wq_sb = singles.tile([128, NET, GD], BF16)  # 16KB/p
            wk_sb = singles.tile([128, NET, D], BF16)  # 4KB/p
            wv_sb = singles.tile([128, NET, D], BF16)  # 4KB/p
            wo_sb = singles.tile([128, G, E], BF16)  # 16KB/p
            tri = singles.tile([128, 128], BF16)  # [k, q] 1=attend
            ident_f = singles.tile([128, 128], F32)
            ident = singles.tile([128, 128], BF16)
            ones1 = singles.tile([128, 1], BF16)
            qT = singles.tile([128, G, S], BF16)  # 16KB/p
            kT = singles.tile([128, S], BF16)  # 4KB/p
            v_sb = singles.tile([128, NKT, D], BF16)  # 4KB/p
            onrm = singles.tile([128, G, S], BF16)  # 16KB/p

            make_identity(nc, ident_f)
            nc.scalar.activation(out=ident[:], in_=ident_f[:], func=AF.Copy)
            nc.vector.memset(ones1, 1.0)

            xts = {}

            def load_chunk(sc, eng=None):
                eng = eng or {}
                for nm, dram in (("xk", xk), ("xv", xv), ("xq", xq)):
                    t = xtp.tile([128, NET, SC], BF16, tag="xt")
                    eng.get(nm, nc.sync).dma_start(
                        out=t[:],
                        in_=tiled3(dram, S, NET, coff=sc * SC, width=SC),
                    )
                    xts[(nm, sc)] = t

            # startup: split the first chunk across the sync queue (xk, xq)
            # and the gpsimd swdge queue (weights, xv) so both queues fill
            # the HBM pipe in parallel; order follows first use.
            t0 = xtp.tile([128, NET, SC], BF16, tag="xt")
            nc.sync.dma_start(
                out=t0[:, : NET // 2, :], in_=tiled3(xk, S, NET // 2, width=SC)
            )
            nc.sync.dma_start(
                out=t0[:, NET // 2 :, :],
                in_=bass.AP(
                    tensor=xk[:, :].tensor,
                    offset=(NET // 2) * 128 * S,
                    ap=[[S, 128], [128 * S, NET // 2], [1, SC]],
                ),
            )
            xts[("xk", 0)] = t0
            nc.gpsimd.dma_start(out=wk_sb[:], in_=tiled3(wk, D, NET))
            t0 = xtp.tile([128, NET, SC], BF16, tag="xt")
            nc.gpsimd.dma_start(out=t0[:], in_=tiled3(xv, S, NET, width=SC))
            xts[("xv", 0)] = t0
            t0 = xtp.tile([128, NET, SC], BF16, tag="xt")
            nc.sync.dma_start(out=t0[:], in_=tiled3(xq, S, NET, width=SC))
            xts[("xq", 0)] = t0
            nc.gpsimd.dma_start(out=wv_sb[:], in_=tiled3(wv, D, NET))
            nc.gpsimd.dma_start(out=wq_sb[:], in_=tiled3(wq, GD, NET))
            nc.gpsimd.dma_start(out=tri[:], in_=msk[:, :])
            nc.gpsimd.dma_start(out=wo_sb[:], in_=tiled3(wo, E, G))

            def proj(w_sb, x_t, out_ps, hslice):
                for t in range(NET):
                    nc.tensor.matmul(
                        out_ps[:],
                        lhsT=w_sb[:, t, hslice],
                        rhs=x_t[:, t, :],
                        start=(t == 0),
                        stop=(t == NET - 1),
                    )

            def emit_outproj_stripe(c, sti):
                st = c * (SC // 128) + sti
                stl = slice(st * 128, (st + 1) * 128)
                ob = obp.tile([128, E], BF16, tag="ob")
                for ec in range(E // SC):
                    esl = slice(ec * SC, (ec + 1) * SC)
                    ps = acc.tile([128, SC], F32, tag="acc")
                    for h in range(G):
                        nc.tensor.matmul(
                            ps[:],
                            lhsT=onrm[:, h, stl],
                            rhs=wo_sb[:, h, esl],
                            start=(h == 0),
                            stop=(h == G - 1),
                        )
                    nc.vector.tensor_copy(out=ob[:, esl], in_=ps[:])
                nc.gpsimd.dma_start(out=out[stl, :], in_=ob[:])

            def emit_normalize(h, ssl):
                # 1/l on [1,SC]; bounce through DRAM (gpsimd swdge queue)
                # to broadcast across partitions; multiply out of PSUM.
                # (gpsimd partition_broadcast measured WORSE: the custom-ISA
                # op thrashes pool microcode libraries between op types)
                o_ps, l_ps = olps[h]
                rl = rlp.tile([1, SC], F32, tag="rl")
                nc.vector.reciprocal_approx_fast(out=rl[:], in_=l_ps[:])
                l_dr = drp.tile([1, SC], F32, tag="ldr")
                nc.gpsimd.dma_start(out=l_dr[:], in_=rl[:])
                rb = rbp.tile([128, SC], F32, tag="rb")
                l_bc = bass.AP(
                    tensor=l_dr[:].tensor,
                    offset=l_dr[:].offset,
                    ap=[[0, 128]] + list(l_dr[:].ap[1:]),
                )
                nc.gpsimd.dma_start(out=rb[:], in_=l_bc)
                nc.vector.tensor_mul(onrm[:, h, ssl], o_ps[:], rb[:])

            olps = {}
            for sc in range(NSC):
                ssl = slice(sc * SC, (sc + 1) * SC)
                if sc + 1 < NSC:
                    load_chunk(sc + 1)
                # K projection
                ps = acc.tile([128, SC], F32, tag="acc")
                proj(wk_sb, xts[("xk", sc)], ps, slice(0, D))
                nc.vector.tensor_copy(out=kT[:, ssl], in_=ps[:])
                # V projection + transpose to [s, d]
                ps = acc.tile([128, SC], F32, tag="acc")
                proj(wv_sb, xts[("xv", sc)], ps, slice(0, D))
                # vt copy on DVE: the scalar queue backs up ~20us of exps
                # at chunk boundaries and would stall the transposes
                vt = vtp.tile([128, SC], BF16, tag="vt")
                nc.vector.tensor_copy(out=vt[:], in_=ps[:])
                for i in range(SC // 128):
                    tp = trp.tile([128, 128], BF16, tag="tr")
                    nc.tensor.transpose(
                        tp[:], vt[:, i * 128 : (i + 1) * 128], ident[:]
                    )
                    nc.vector.tensor_copy(out=v_sb[:, sc * 4 + i, :], in_=tp[:])

                nkt = (sc + 1) * (SC // 128)  # causal: k tiles 0..nkt-1
                for h in range(G):
                    # Q projection for this head
                    ps = acc.tile([128, SC], F32, tag="acc")
                    proj(wq_sb, xts[("xq", sc)], ps, slice(h * D, (h + 1) * D))
                    nc.vector.tensor_copy(out=qT[:, h, ssl], in_=ps[:])

                    # deferred: normalize previous head + outproj stripe of
                    # the previous chunk (keeps the DVE queue clear of the
                    # DMA-bounce wait when the next head's scores start)
                    if h > 0:
                        emit_normalize(h - 1, ssl)
                        if sc > 0:
                            emit_outproj_stripe(sc - 1, h - 1)

                    # attention: scores run one k-tile ahead of AV/l
                    o_ps = ops.tile([128, SC], F32, tag="o")
                    l_ps = lps.tile([1, SC], F32, tag="l")
                    olps[h] = (o_ps, l_ps)
                    pps = [None] * nkt

                    def emit_scores(kt):
                        pp = pexp.tile([128, SC], BF16, tag="p")
                        pps[kt] = pp
                        jj = kt - (nkt - 4)
                        w0 = max(jj, 0) * 128  # first unmasked column
                        s_ps = acc.tile([128, SC], F32, tag="acc")
                        nc.tensor.matmul(
                            s_ps[:, w0:SC],
                            lhsT=kT[:, kt * 128 : (kt + 1) * 128],
                            rhs=qT[:, h, sc * SC + w0 : (sc + 1) * SC],
                            start=True,
                            stop=True,
                        )
                        nc.scalar.activation(
                            out=pp[:, w0:SC],
                            in_=s_ps[:, w0:SC],
                            func=AF.Exp,
                            scale=SCALE,
                        )
                        if jj >= 0:
                            dsl = slice(jj * 128, (jj + 1) * 128)
                            nc.gpsimd.tensor_mul(pp[:, dsl], pp[:, dsl], tri[:])

                    def emit_av(kt):
                        # masked (zero) columns of diagonal tiles are
                        # skipped: kt==0 is always full width so start=True
                        # initializes the whole PSUM region.
                        pp = pps[kt]
                        w0 = max(kt - (nkt - 4), 0) * 128
                        nc.tensor.matmul(
                            o_ps[:, w0:SC],
                            lhsT=v_sb[:, kt, :],
                            rhs=pp[:, w0:SC],
                            start=(kt == 0),
                            stop=(kt == nkt - 1),
                        )
                        nc.tensor.matmul(
                            l_ps[:, w0:SC],
                            lhsT=ones1[:],
                            rhs=pp[:, w0:SC],
                            start=(kt == 0),
                            stop=(kt == nkt - 1),
                        )

                    for kt in range(nkt):
                        emit_scores(kt)
                        if kt >= 1:
                            emit_av(kt - 1)
                    emit_av(nkt - 1)

                emit_normalize(G - 1, ssl)
                if sc > 0:
                    emit_outproj_stripe(sc - 1, G - 1)

            for sti in range(SC // 128):
                emit_outproj_stripe(NSC - 1, sti)
    nc.compile()
    return nc


_NC_CACHE = None


def _get_nc():
    global _NC_CACHE
    if _NC_CACHE is None:
        _NC_CACHE = build_nc()
    return _NC_CACHE


def _prep_in_maps(query, key, value, attn_mask, Wq, Wk, Wv, Wo):
    query = np.asarray(query, dtype=np.float32)
    key = np.asarray(key, dtype=np.float32)
    value = np.asarray(value, dtype=np.float32)
    Wq = np.asarray(Wq, dtype=np.float32)
    Wk = np.asarray(Wk, dtype=np.float32)
    Wv = np.asarray(Wv, dtype=np.float32)
    Wo = np.asarray(Wo, dtype=np.float32)
    am = np.asarray(attn_mask)

    xqT = [np.ascontiguousarray(query[b].T).astype(NPBF) for b in range(B)]
    xkT = [np.ascontiguousarray(key[b].T).astype(NPBF) for b in range(B)]
    xvT = [np.ascontiguousarray(value[b].T).astype(NPBF) for b in range(B)]

    # [k, q] multiplicative triangle for the diagonal 128x128 blocks
    m0 = np.asarray(am[0, 0, :128, :128], dtype=np.float32)  # [q, k]
    tri = np.ascontiguousarray(m0.T).astype(NPBF)

    in_maps = []
    for b in range(B):
        for g in range(HKV):
            in_maps.append(
                {
                    "xq": xqT[b],
                    "xk": xkT[b],
                    "xv": xvT[b],
                    "wq": np.ascontiguousarray(
                        Wq[g * GD : (g + 1) * GD, :].T
                    ).astype(NPBF),
                    "wk": np.ascontiguousarray(
                        Wk[g * D : (g + 1) * D, :].T
                    ).astype(NPBF),
                    "wv": np.ascontiguousarray(
                        Wv[g * D : (g + 1) * D, :].T
                    ).astype(NPBF),
                    "wo": np.ascontiguousarray(
                        Wo[:, g * GD : (g + 1) * GD].T
                    ).astype(NPBF),
                    "msk": tri,
                }
            )
    return in_maps


def _run(inputs, trace=False, **kw):
    nc = _get_nc()
    in_maps = _prep_in_maps(**inputs)
    res = run_bass_kernel_spmd(
        nc, in_maps, list(range(NCORES)), trace=trace, **kw
    )
    outs = [np.asarray(r["out"]) for r in res.results]
    full = np.empty((B, S, E), dtype=np.float32)
    for b in range(B):
        acc = outs[b * HKV].astype(np.float32)
        for g in range(1, HKV):
            acc = acc + outs[b * HKV + g].astype(np.float32)
        full[b] = acc
    return full, res


def kernel(**inputs):
    full, _ = _run(inputs, trace=False)
    return full


# revision 19
# speedup vs baseline: 1.3890x; 1.0206x over previous
"""GQA kernel for Trainium2, 8 NeuronCores.

Sharding: core c = b*4 + g handles batch b, kv-head g (4 query heads).
Host sums the 4 partial outputs per batch.

Per core (all matmuls bf16, f32 PSUM):
  Q_h^T = Wq_h @ x_q^T   [128 d, S]
  K^T   = Wk  @ x_k^T    [128 d, S]
  V     = [S, 128]  (V^T via matmul, then PE transpose)
  S^T   = K_tile @ Q^T -> [k, q] scores; exp on scalar engine into
          bf16 P; causal diagonal handled by column-narrowed scores
          matmul + narrowed exp + gpsimd memset (fully-masked cols)
          + gpsimd triangle multiply (keeps DVE/scalar free)
  o^T  += V[kt] @ P[kt] (PSUM), l += ones @ P[kt]
  norm  = reciprocal_approx_fast(l) [1,SC] -> DRAM bounce broadcast
          (gpsimd swdge queue) -> DVE mul out of PSUM
  out   = onrm @ Wo -> bf16 partial [S, E]

Perf notes vs the 477us baseline:
 - The PE clock p-states ramp 0.65 -> 1.2 -> 2.4 GHz and reach full
   speed only after ~3us of gap-free execution, so emission is
   software-pipelined: per attention head we interleave the next
   head's Q projection and the previous chunk's outproj stripes so
   the PE queue never starves while exps run.
 - The l reciprocal runs on [1,SC] via reciprocal_approx_fast (~5x
   cheaper than InstReciprocal on the broadcast [128,SC]).
 - DMAs are batched (one 3D-AP descriptor per x chunk / weight) and
   spread across the sync queue (x, V transposes) and gpsimd SWDGE
   (weights, l bounce, output writes) so no single queue serializes
   and DMA issue stays off the scalar/vector engines.
 - exp work on the causal diagonal is column-narrowed (~12% less
   scalar time); triangle masking runs on the idle gpsimd engine as
   a 0/1 multiply after exp.
"""

import sys

import numpy as np

for _p in ("/opt/trn_rl_repo",):
    if _p not in sys.path:
        sys.path.insert(0, _p)

import ml_dtypes

import concourse.bass as bass
import concourse.mybir as mybir
from concourse import bacc
from concourse.bass_utils import run_bass_kernel_spmd
from concourse.masks import make_identity
from concourse.tile import TileContext

B, S, E = 2, 2048, 2048
H, HKV = 16, 4
D = E // H  # 128
G = H // HKV  # 4 query heads per kv head
GD = G * D  # 512
NCORES = B * HKV  # 8
SC = 512  # s/q chunk width (free dim of matmuls)
NSC = S // SC  # 4
NET = E // 128  # 16 e-tiles (contraction)
NKT = S // 128  # 16 k-tiles
SCALE = 1.0 / float(np.sqrt(D))

F32 = mybir.dt.float32
BF16 = mybir.dt.bfloat16
AF = mybir.ActivationFunctionType
NPBF = np.dtype(ml_dtypes.bfloat16)


def build_nc():
    nc = bacc.Bacc()
    xq = nc.declare_dram_parameter("xq", [E, S], BF16, isOutput=False)  # query[b].T
    xk = nc.declare_dram_parameter("xk", [E, S], BF16, isOutput=False)  # key[b].T
    xv = nc.declare_dram_parameter("xv", [E, S], BF16, isOutput=False)  # value[b].T
    wq = nc.declare_dram_parameter("wq", [E, GD], BF16, isOutput=False)
    wk = nc.declare_dram_parameter("wk", [E, D], BF16, isOutput=False)
    wv = nc.declare_dram_parameter("wv", [E, D], BF16, isOutput=False)
    wo = nc.declare_dram_parameter("wo", [GD, E], BF16, isOutput=False)
    msk = nc.declare_dram_parameter("msk", [128, 128], BF16, isOutput=False)
    out = nc.declare_dram_parameter("out", [S, E], BF16, isOutput=True)

    def tiled3(dram, ncols, tiles, coff=0, width=None):
        # [128, tiles, width] gather of row-tiled [tiles*128, ncols] DRAM
        w = ncols if width is None else width
        base = dram[:, :]
        return bass.AP(
            tensor=base.tensor,
            offset=coff,
            ap=[[ncols, 128], [128 * ncols, tiles], [1, w]],
        )

    with TileContext(nc) as tc:
        with (
            tc.tile_pool(name="singles", bufs=1) as singles,
            tc.tile_pool(name="xt", bufs=5) as xtp,
            tc.tile_pool(name="pexp", bufs=4) as pexp,
            tc.tile_pool(name="vt", bufs=2) as vtp,
            tc.tile_pool(name="rl", bufs=2) as rlp,
            tc.tile_pool(name="rb", bufs=2) as rbp,
            tc.tile_pool(name="ob", bufs=2) as obp,
            tc.tile_pool(name="acc", bufs=4, space="PSUM") as acc,
            tc.tile_pool(name="ops", bufs=2, space="PSUM") as ops,
            tc.tile_pool(name="lps", bufs=1, space="PSUM") as lps,
            tc.tile_pool(name="trp", bufs=1, space="PSUM") as trp,
            tc.tile_pool(name="drp", bufs=2, space="DRAM") as drp,
        ):
            # ---- constants / weights resident in SBUF ----
            wq_sb = singles.tile([128, NET, GD], BF16)  # 16KB/p
            wk_sb = singles.tile([128, NET, D], BF16)  # 4KB/p
            wv_sb = singles.tile([128, NET, D], BF16)  # 4KB/p
            wo_sb = singles.tile([128, G, E], BF16)  # 16KB/p
            tri = singles.tile([128, 128], BF16)  # [k, q] 1=attend
            ident_f = singles.tile([128, 128], F32)
            ident = singles.tile([128, 128], BF16)
            ones1 = singles.tile([128, 1], BF16)
            qT = singles.tile([128, G, S], BF16)  # 16KB/p
            kT = singles.tile([128, S], BF16)  # 4KB/p
            v_sb = singles.tile([128, NKT, D], BF16)  # 4KB/p
            onrm = singles.tile([128, G, S], BF16)  # 16KB/p

            make_identity(nc, ident_f)
            nc.scalar.activation(out=ident[:], in_=ident_f[:], func=AF.Copy)
            nc.vector.memset(ones1, 1.0)

            xts = {}

            def load_chunk(sc, eng=None):
                eng = eng or {}
                for nm, dram in (("xk", xk), ("xv", xv), ("xq", xq)):
                    t = xtp.tile([128, NET, SC], BF16, tag="xt")
                    eng.get(nm, nc.sync).dma_start(
                        out=t[:],
                        in_=tiled3(dram, S, NET, coff=sc * SC, width=SC),
                    )
                    xts[(nm, sc)] = t

            # startup: weights on the scalar hwdge queue (in parallel with
            # the x loads on sync; swdge is useless for these strided
            # patterns: gpsimd pays ~per-descriptor microcode cost). Order
            # follows first use.
            nc.scalar.dma_start(out=wk_sb[:], in_=tiled3(wk, D, NET))
            t0 = xtp.tile([128, NET, SC], BF16, tag="xt")
            nc.sync.dma_start(
                out=t0[:, : NET // 2, :], in_=tiled3(xk, S, NET // 2, width=SC)
            )
            nc.sync.dma_start(
                out=t0[:, NET // 2 :, :],
                in_=bass.AP(
                    tensor=xk[:, :].tensor,
                    offset=(NET // 2) * 128 * S,
                    ap=[[S, 128], [128 * S, NET // 2], [1, SC]],
                ),
            )
            xts[("xk", 0)] = t0
            nc.scalar.dma_start(out=wv_sb[:], in_=tiled3(wv, D, NET))
            t0 = xtp.tile([128, NET, SC], BF16, tag="xt")
            nc.sync.dma_start(out=t0[:], in_=tiled3(xv, S, NET, width=SC))
            xts[("xv", 0)] = t0
            nc.scalar.dma_start(out=wq_sb[:], in_=tiled3(wq, GD, NET))
            t0 = xtp.tile([128, NET, SC], BF16, tag="xt")
            nc.sync.dma_start(out=t0[:], in_=tiled3(xq, S, NET, width=SC))
            xts[("xq", 0)] = t0
            nc.scalar.dma_start(out=tri[:], in_=msk[:, :])
            nc.scalar.dma_start(out=wo_sb[:], in_=tiled3(wo, E, G))

            def proj(w_sb, x_t, out_ps, hslice):
                for t in range(NET):
                    nc.tensor.matmul(
                        out_ps[:],
                        lhsT=w_sb[:, t, hslice],
                        rhs=x_t[:, t, :],
                        start=(t == 0),
                        stop=(t == NET - 1),
                    )

            def emit_outproj_stripe(c, sti):
                st = c * (SC // 128) + sti
                stl = slice(st * 128, (st + 1) * 128)
                ob = obp.tile([128, E], BF16, tag="ob")
                for ec in range(E // SC):
                    esl = slice(ec * SC, (ec + 1) * SC)
                    ps = acc.tile([128, SC], F32, tag="acc")
                    for h in range(G):
                        nc.tensor.matmul(
                            ps[:],
                            lhsT=onrm[:, h, stl],
                            rhs=wo_sb[:, h, esl],
                            start=(h == 0),
                            stop=(h == G - 1),
                        )
                    nc.vector.tensor_copy(out=ob[:, esl], in_=ps[:])
                nc.gpsimd.dma_start(out=out[stl, :], in_=ob[:])

            def emit_normalize(h, ssl):
                # 1/l on [1,SC]; bounce through DRAM (gpsimd swdge queue)
                # to broadcast across partitions; multiply out of PSUM.
                # (gpsimd partition_broadcast measured WORSE: the custom-ISA
                # op thrashes pool microcode libraries between op types)
                o_ps, l_ps = olps[h]
                rl = rlp.tile([1, SC], F32, tag="rl")
                nc.vector.reciprocal_approx_fast(out=rl[:], in_=l_ps[:])
                l_dr = drp.tile([1, SC], F32, tag="ldr")
                nc.gpsimd.dma_start(out=l_dr[:], in_=rl[:])
                rb = rbp.tile([128, SC], F32, tag="rb")
                l_bc = bass.AP(
                    tensor=l_dr[:].tensor,
                    offset=l_dr[:].offset,
                    ap=[[0, 128]] + list(l_dr[:].ap[1:]),
                )
                # rb read on the scalar hwdge queue: hardware descriptor
                # generation; swdge needs ~128 microcoded descriptors here
                nc.scalar.dma_start(out=rb[:], in_=l_bc)
                nc.vector.tensor_mul(onrm[:, h, ssl], o_ps[:], rb[:])

            olps = {}
            for sc in range(NSC):
                ssl = slice(sc * SC, (sc + 1) * SC)
                if sc + 1 < NSC:
                    load_chunk(sc + 1)
                # K projection
                ps = acc.tile([128, SC], F32, tag="acc")
                proj(wk_sb, xts[("xk", sc)], ps, slice(0, D))
                nc.vector.tensor_copy(out=kT[:, ssl], in_=ps[:])
                # V projection + transpose to [s, d]
                ps = acc.tile([128, SC], F32, tag="acc")
                proj(wv_sb, xts[("xv", sc)], ps, slice(0, D))
                # vt copy on DVE: the scalar queue backs up ~20us of exps
                # at chunk boundaries and would stall the transposes
                vt = vtp.tile([128, SC], BF16, tag="vt")
                nc.vector.tensor_copy(out=vt[:], in_=ps[:])
                for i in range(SC // 128):
                    tp = trp.tile([128, 128], BF16, tag="tr")
                    nc.tensor.transpose(
                        tp[:], vt[:, i * 128 : (i + 1) * 128], ident[:]
                    )
                    nc.vector.tensor_copy(out=v_sb[:, sc * 4 + i, :], in_=tp[:])

                nkt = (sc + 1) * (SC // 128)  # causal: k tiles 0..nkt-1
                for h in range(G):
                    # Q projection for this head
                    ps = acc.tile([128, SC], F32, tag="acc")
                    proj(wq_sb, xts[("xq", sc)], ps, slice(h * D, (h + 1) * D))
                    nc.vector.tensor_copy(out=qT[:, h, ssl], in_=ps[:])

                    # deferred: normalize previous head + outproj stripe of
                    # the previous chunk (keeps the DVE queue clear of the
                    # DMA-bounce wait when the next head's scores start)
                    if h > 0:
                        emit_normalize(h - 1, ssl)
                        if sc > 0:
                            emit_outproj_stripe(sc - 1, h - 1)

                    # attention: scores run one k-tile ahead of AV/l
                    o_ps = ops.tile([128, SC], F32, tag="o")
                    l_ps = lps.tile([1, SC], F32, tag="l")
                    olps[h] = (o_ps, l_ps)
                    pps = [None] * nkt

                    def emit_scores(kt):
                        pp = pexp.tile([128, SC], BF16, tag="p")
                        pps[kt] = pp
                        jj = kt - (nkt - 4)
                        w0 = max(jj, 0) * 128  # first unmasked column
                        s_ps = acc.tile([128, SC], F32, tag="acc")
                        nc.tensor.matmul(
                            s_ps[:, w0:SC],
                            lhsT=kT[:, kt * 128 : (kt + 1) * 128],
                            rhs=qT[:, h, sc * SC + w0 : (sc + 1) * SC],
                            start=True,
                            stop=True,
                        )
                        nc.scalar.activation(
                            out=pp[:, w0:SC],
                            in_=s_ps[:, w0:SC],
                            func=AF.Exp,
                            scale=SCALE,
                        )
                        if jj >= 0:
                            dsl = slice(jj * 128, (jj + 1) * 128)
                            nc.gpsimd.tensor_mul(pp[:, dsl], pp[:, dsl], tri[:])

                    def emit_av(kt):
                        # masked (zero) columns of diagonal tiles are
                        # skipped: kt==0 is always full width so start=True
                        # initializes the whole PSUM region.
                        pp = pps[kt]
                        w0 = max(kt - (nkt - 4), 0) * 128
                        nc.tensor.matmul(
                            o_ps[:, w0:SC],
                            lhsT=v_sb[:, kt, :],
                            rhs=pp[:, w0:SC],
                            start=(kt == 0),
                            stop=(kt == nkt - 1),
                        )
                        nc.tensor.matmul(
                            l_ps[:, w0:SC],
                            lhsT=ones1[:],
                            rhs=pp[:, w0:SC],
                            start=(kt == 0),
                            stop=(kt == nkt - 1),
                        )

                    for kt in range(nkt):
                        emit_scores(kt)
                        if kt >= 1:
                            emit_av(kt - 1)
                    emit_av(nkt - 1)

                emit_normalize(G - 1, ssl)
                if sc > 0:
                    emit_outproj_stripe(sc - 1, G - 1)

            for sti in range(SC // 128):
                emit_outproj_stripe(NSC - 1, sti)
    nc.compile()
    return nc


_NC_CACHE = None


def _get_nc():
    global _NC_CACHE
    if _NC_CACHE is None:
        _NC_CACHE = build_nc()
    return _NC_CACHE


def _prep_in_maps(query, key, value, attn_mask, Wq, Wk, Wv, Wo):
    query = np.asarray(query, dtype=np.float32)
    key = np.asarray(key, dtype=np.float32)
    value = np.asarray(value, dtype=np.float32)
    Wq = np.asarray(Wq, dtype=np.float32)
    Wk = np.asarray(Wk, dtype=np.float32)
    Wv = np.asarray(Wv, dtype=np.float32)
    Wo = np.asarray(Wo, dtype=np.float32)
    am = np.asarray(attn_mask)

    xqT = [np.ascontiguousarray(query[b].T).astype(NPBF) for b in range(B)]
    xkT = [np.ascontiguousarray(key[b].T).astype(NPBF) for b in range(B)]
    xvT = [np.ascontiguousarray(value[b].T).astype(NPBF) for b in range(B)]

    # [k, q] multiplicative triangle for the diagonal 128x128 blocks
    m0 = np.asarray(am[0, 0, :128, :128], dtype=np.float32)  # [q, k]
    tri = np.ascontiguousarray(m0.T).astype(NPBF)

    in_maps = []
    for b in range(B):
        for g in range(HKV):
            in_maps.append(
                {
                    "xq": xqT[b],
                    "xk": xkT[b],
                    "xv": xvT[b],
                    "wq": np.ascontiguousarray(
                        Wq[g * GD : (g + 1) * GD, :].T
                    ).astype(NPBF),
                    "wk": np.ascontiguousarray(
                        Wk[g * D : (g + 1) * D, :].T
                    ).astype(NPBF),
                    "wv": np.ascontiguousarray(
                        Wv[g * D : (g + 1) * D, :].T
                    ).astype(NPBF),
                    "wo": np.ascontiguousarray(
                        Wo[:, g * GD : (g + 1) * GD].T
                    ).astype(NPBF),
                    "msk": tri,
                }
            )
    return in_maps


def _run(inputs, trace=False, **kw):
    nc = _get_nc()
    in_maps = _prep_in_maps(**inputs)
    res = run_bass_kernel_spmd(
        nc, in_maps, list(range(NCORES)), trace=trace, **kw
    )
    outs = [np.asarray(r["out"]) for r in res.results]
    full = np.empty((B, S, E), dtype=np.float32)
    for b in range(B):
        acc = outs[b * HKV].astype(np.float32)
        for g in range(1, HKV):
            acc = acc + outs[b * HKV + g].astype(np.float32)
        full[b] = acc
    return full, res


def kernel(**inputs):
    full, _ = _run(inputs, trace=False)
    return full
